# revision 13
# baseline (speedup 1.0000x reference)
"""Trainium2 Bass kernel for the ExpInstantaneousPhase loss.

Math:
    part1 + part2 = 2 + 2*(x*y + Hx*Hy)/(Ax*Ay)   (Ax^2 = x^2 + Hx^2)
    loss = -2*B - (2/N) * sum(e),  e = (x*y + Hx*Hy) * rsqrt((x^2+Hx^2)*(y^2+Hy^2))
where H is the Hilbert transform along the 3000-sample time axis. Both Hilberts
come from one complex FFT round-trip of z = x + i*y:
    a = IFFT(h * FFT(z)),  Hx = Im(a) - y,  Hy = x - Re(a)
The size-3000 FFT+mask+IFFT is factored (3000 = 120*25) into THREE batched
matmul stages executed on the tensor engine (twiddles and the h mask folded
into the per-group weight matrices):
    S1: per n2 in [0,25):  120x120 complex mats, contract n1
    S2: per k1 in [0,120): 25x25 complex mats D_k1 (h absorbed), contract n2
        (5 k1-values block-diag packed into 125x125)
    S3: per m2 in [0,25):  120x120 complex mats, contract k1
Sharding: shot s -> core s (8 shots, 8 cores); each core handles 1200 signals.
"""

import numpy as np
import ml_dtypes

N = 3000
N1, N2 = 120, 25
S = 1200          # signals per core
C = 300           # signal-chunk width
NCH = S // C      # 4 chunks
HALF = C // 2     # elementwise half-chunk
NCORES = 8
MAGIC = 0x5F35    # bf16 rsqrt bit-trick constant (calibrated)
BF = ml_dtypes.bfloat16

_CACHE = {}
TRACE = False
TRACE_KW = {}
STAGES = 99  # debug: 1=S1 only, 2=+turn1/S2, 3=+turn2/S3, 4=+elementwise


def _build_weights():
    """Host-side: the three stages' lhsT weight stacks, bf16."""
    w = np.exp(-2j * np.pi / N)
    k1 = np.arange(N1)
    n2 = np.arange(N2)
    k2 = np.arange(N2)
    h = np.zeros(N)
    h[0] = 1.0
    h[N // 2] = 1.0
    h[1:N // 2] = 2.0

    W1 = np.exp(-2j * np.pi * np.outer(k1, k1) / N1)        # [k1, n1]
    S1m = np.array([(w ** (j * k1))[:, None] * W1 for j in range(N2)])   # [25,120,120]
    WN2 = np.exp(-2j * np.pi * np.outer(k2, n2) / N2)       # [k2, n2]
    WN2i = np.exp(+2j * np.pi * np.outer(n2, k2) / N2)      # [m2, k2]
    D = np.array([WN2i @ (h[a + N1 * k2][:, None] * WN2) for a in range(N1)])  # [120,25,25]
    W1i = np.exp(+2j * np.pi * np.outer(k1, k1) / N1)       # [m1, k1]
    S3m = np.array([(1.0 / N) * W1i * (w ** (-j * k1))[None, :] for j in range(N2)])  # [25,120,120]

    def lhst3(mat):  # [r, i, negi] lhsT stack from complex mat (out_dim, in_dim)
        t = mat.T  # lhsT = [contract, out]
        return np.stack([t.real, t.imag, -t.imag])

    w1 = np.stack([lhst3(S1m[j]) for j in range(N2)])       # [25, 3, 120, 120]
    w3 = np.stack([lhst3(S3m[j]) for j in range(N2)])       # [25, 3, 120, 120]

    # S2: block-diag pack of 5 k1 per group, k1 = 5*g + j  (j in [0,5))
    # input basis p = 25*j + n2 ; output basis q = 5*m2 + j
    w2 = np.zeros((24, 3, 125, 125))
    for g in range(24):
        blk = np.zeros((125, 125), dtype=complex)
        for j in range(5):
            for m2 in range(N2):
                blk[5 * m2 + j, 25 * j:25 * j + 25] = D[5 * g + j][m2, :]
        w2[g] = lhst3(blk)

    # S3 contraction basis r = 24*j + g  ->  k1 = 5*(r % 24) + r // 24
    perm = np.array([5 * (r % 24) + r // 24 for r in range(N1)])
    w3 = w3[:, :, perm, :]
    return w1.astype(BF), w2.astype(BF), w3.astype(BF)


def _build_nc():
    import concourse.bacc as bacc
    import concourse.mybir as mybir
    from concourse.tile import TileContext

    fp32 = mybir.dt.float32
    bf16 = mybir.dt.bfloat16
    i16 = mybir.dt.int16
    ALU = mybir.AluOpType

    nc = bacc.Bacc(None, target_bir_lowering=False)
    x_d = nc.dram_tensor("x", [N, S], bf16, kind="ExternalInput")
    y_d = nc.dram_tensor("y", [N, S], bf16, kind="ExternalInput")
    w1_d = nc.dram_tensor("w1", [N2, 3, N1, N1], bf16, kind="ExternalInput")
    w2_d = nc.dram_tensor("w2", [24, 3, 125, 125], bf16, kind="ExternalInput")
    w3_d = nc.dram_tensor("w3", [N2, 3, N1, N1], bf16, kind="ExternalInput")
    acc_d = nc.dram_tensor("acc", [NCH * 2, N1], fp32, kind="ExternalOutput")

    with TileContext(nc) as tc:
        with (
            tc.tile_pool(name="consts", bufs=1) as consts,
            tc.tile_pool(name="io", bufs=1) as io,
            tc.tile_pool(name="big", bufs=1) as big,
            tc.tile_pool(name="turn", bufs=1) as turn,
            tc.tile_pool(name="ew", bufs=1) as ew,
            tc.tile_pool(name="accp", bufs=NCH * 2) as accp,
            tc.tile_pool(name="psum", bufs=3, space="PSUM") as psum,
        ):
            # --- load weights (once) ---
            w1_sb = consts.tile([N1, N2, 3, N1], bf16)
            nc.sync.dma_start(out=w1_sb, in_=w1_d.rearrange("n m a b -> a n m b"))
            w2_sb = consts.tile([125, 24, 3, 125], bf16)
            nc.sync.dma_start(out=w2_sb, in_=w2_d.rearrange("n m a b -> a n m b"))
            w3_sb = consts.tile([N1, N2, 3, N1], bf16)
            nc.sync.dma_start(out=w3_sb, in_=w3_d.rearrange("n m a b -> a n m b"))

            x_r = x_d.rearrange("(a b) s -> a b s", b=N2)   # [120, 25, 1200]
            y_r = y_d.rearrange("(a b) s -> a b s", b=N2)

            for ch in range(NCH):
                c0 = ch * C
                xb = io.tile([N1, N2, C], bf16, tag="xb")
                yb = io.tile([N1, N2, C], bf16, tag="yb")
                nc.sync.dma_start(out=xb, in_=x_r[:, :, c0:c0 + C])
                nc.sync.dma_start(out=yb, in_=y_r[:, :, c0:c0 + C])

                # ---- S1: contract n1 (120) per n2 ----
                A = big.tile([N1, 2, N2, C], bf16, tag="big")
                for j in range(N2):
                    ps = psum.tile([128, 1024], fp32, tag="ps")
                    pv = ps.rearrange("p (b w) -> p b w", b=2)
                    wr = w1_sb[:, j, 0, :]
                    wi = w1_sb[:, j, 1, :]
                    wn = w1_sb[:, j, 2, :]
                    nc.tensor.matmul(pv[:N1, 0, :C], wr, xb[:, j, :], start=True, stop=False)
                    nc.tensor.matmul(pv[:N1, 1, :C], wr, yb[:, j, :], start=True, stop=False)
                    nc.tensor.matmul(pv[:N1, 1, :C], wi, xb[:, j, :], start=False, stop=True)
                    nc.tensor.matmul(pv[:N1, 0, :C], wn, yb[:, j, :], start=False, stop=True)
                    nc.scalar.copy(out=A[:, :, j, :], in_=pv[:N1, :, :C])

                if STAGES < 2:
                    for hh in range(2):
                        acc_t = accp.tile([N1, 1], fp32, tag="acc")
                        nc.vector.memset(acc_t, 0.0)
                        nc.sync.dma_start(out=acc_d[ch * 2 + hh, :], in_=acc_t[:, 0:1])
                    continue

                # ---- corner turn 1: [k1=5g+j, n2] -> p = 25j + n2 per group g ----
                A2 = turn.tile([125, 2, 24, C], bf16, tag="turn")
                for g in range(24):
                    for pl in range(2):
                        nc.sync.dma_start(out=A2[:, pl, g, :],
                                          in_=A[5 * g:5 * g + 5, pl, :, :])

                # ---- S2: contract n2 (25) per k1, 5-packed ----
                Csb = big.tile([125, 2, 24, C], bf16, tag="big")
                for g in range(24):
                    ps = psum.tile([128, 1024], fp32, tag="ps")
                    pv = ps.rearrange("p (b w) -> p b w", b=2)
                    dr = w2_sb[:, g, 0, :]
                    di = w2_sb[:, g, 1, :]
                    dn = w2_sb[:, g, 2, :]
                    nc.tensor.matmul(pv[:125, 0, :C], dr, A2[:, 0, g, :], start=True, stop=False)
                    nc.tensor.matmul(pv[:125, 1, :C], dr, A2[:, 1, g, :], start=True, stop=False)
                    nc.tensor.matmul(pv[:125, 1, :C], di, A2[:, 0, g, :], start=False, stop=True)
                    nc.tensor.matmul(pv[:125, 0, :C], dn, A2[:, 1, g, :], start=False, stop=True)
                    nc.scalar.copy(out=Csb[:, :, g, :], in_=pv[:125, :, :C])

                if STAGES < 3:
                    for hh in range(2):
                        acc_t = accp.tile([N1, 1], fp32, tag="acc")
                        nc.vector.memset(acc_t, 0.0)
                        nc.sync.dma_start(out=acc_d[ch * 2 + hh, :], in_=acc_t[:, 0:1])
                    continue

                # ---- corner turn 2: [q=5m2+j, g] -> r = 24j + g per m2 ----
                C2 = turn.tile([N1, 2, N2, C], bf16, tag="turn")
                for m2 in range(N2):
                    for pl in range(2):
                        nc.sync.dma_start(out=C2[:, pl, m2, :],
                                          in_=Csb[5 * m2:5 * m2 + 5, pl, :, :])

                # ---- S3: contract k1 (120) per m2 ----
                a3 = big.tile([N1, 2, N2, C], bf16, tag="big")
                for m2 in range(N2):
                    ps = psum.tile([128, 1024], fp32, tag="ps")
                    pv = ps.rearrange("p (b w) -> p b w", b=2)
                    wr = w3_sb[:, m2, 0, :]
                    wi = w3_sb[:, m2, 1, :]
                    wn = w3_sb[:, m2, 2, :]
                    nc.tensor.matmul(pv[:N1, 0, :C], wr, C2[:, 0, m2, :], start=True, stop=False)
                    nc.tensor.matmul(pv[:N1, 1, :C], wr, C2[:, 1, m2, :], start=True, stop=False)
                    nc.tensor.matmul(pv[:N1, 1, :C], wi, C2[:, 0, m2, :], start=False, stop=True)
                    nc.tensor.matmul(pv[:N1, 0, :C], wn, C2[:, 1, m2, :], start=False, stop=True)
                    nc.scalar.copy(out=a3[:, :, m2, :], in_=pv[:N1, :, :C])

                if STAGES < 4:
                    for hh in range(2):
                        acc_t = accp.tile([N1, 1], fp32, tag="acc")
                        nc.vector.memset(acc_t, 0.0)
                        nc.sync.dma_start(out=acc_d[ch * 2 + hh, :], in_=acc_t[:, 0:1])
                    continue

                # ---- elementwise + reduce, two half-chunks ----
                for hh in range(2):
                    sl = slice(hh * HALF, (hh + 1) * HALF)
                    xh = xb[:, :, sl]
                    yh = yb[:, :, sl]
                    arh = a3[:, 0, :, sl]
                    aih = a3[:, 1, :, sl]
                    hy = ew.tile([N1, N2, HALF], bf16, tag="hy")
                    hx = ew.tile([N1, N2, HALF], bf16, tag="hx")
                    nc.vector.tensor_sub(hy, xh, arh)
                    nc.vector.tensor_sub(hx, aih, yh)
                    tp = ew.tile([N1, N2, HALF], bf16, tag="tp")
                    tq = ew.tile([N1, N2, HALF], bf16, tag="tq")
                    nc.vector.tensor_mul(tp, xh, yh)
                    nc.vector.tensor_mul(tq, hx, hy)
                    nc.vector.tensor_add(tp, tp, tq)           # num
                    t1 = ew.tile([N1, N2, HALF], bf16, tag="t1")
                    t2 = ew.tile([N1, N2, HALF], bf16, tag="t2")
                    nc.scalar.square(out=t1, in_=xh)
                    nc.scalar.square(out=t2, in_=hx)
                    nc.vector.tensor_add(t1, t1, t2)           # u = x^2 + hx^2
                    nc.scalar.square(out=t2, in_=yh)
                    nc.scalar.square(out=tq, in_=hy)
                    nc.vector.tensor_add(t2, t2, tq)           # v = y^2 + hy^2
                    nc.vector.tensor_mul(t1, t1, t2)           # w = u*v
                    acc_t = accp.tile([N1, 1], fp32, tag="acc")
                    if STAGES < 5:
                        nc.vector.memset(acc_t, 0.0)
                        nc.sync.dma_start(out=acc_d[ch * 2 + hh, :], in_=acc_t[:, 0:1])
                        continue
                    # rsqrt bit trick: r_bits = MAGIC - (w_bits >> 1)
                    t1i = t1.bitcast(i16)
                    t2i = t2.bitcast(i16)
                    nc.vector.tensor_scalar(t2i, t1i, 1, None, op0=ALU.logical_shift_right)
                    nc.vector.tensor_scalar(t2i, t2i, -1, MAGIC, op0=ALU.mult, op1=ALU.add)
                    if STAGES < 6:
                        nc.vector.memset(acc_t, 0.0)
                        nc.sync.dma_start(out=acc_d[ch * 2 + hh, :], in_=acc_t[:, 0:1])
                        continue
                    nc.vector.scalar_tensor_tensor(
                        out=tq, in0=tp, scalar=1.0, in1=t2,
                        op0=ALU.mult, op1=ALU.mult, accum_out=acc_t)
                    nc.sync.dma_start(out=acc_d[ch * 2 + hh, :], in_=acc_t[:, 0:1])
    nc.finalize()
    return nc


def kernel(x: np.ndarray, y: np.ndarray) -> np.ndarray:
    from concourse.bass_utils import run_bass_kernel_spmd

    if "nc" not in _CACHE:
        _CACHE["nc"] = _build_nc()
        _CACHE["w"] = _build_weights()
    nc = _CACHE["nc"]
    w1, w2, w3 = _CACHE["w"]

    in_maps = []
    for c in range(NCORES):
        xc = np.ascontiguousarray(x[c].reshape(N, S)).astype(BF)
        yc = np.ascontiguousarray(y[c].reshape(N, S)).astype(BF)
        in_maps.append({"x": xc, "y": yc, "w1": w1, "w2": w2, "w3": w3})

    res = run_bass_kernel_spmd(nc, in_maps, list(range(NCORES)),
                               trace=TRACE, **TRACE_KW)
    _CACHE["last"] = res
    total = 0.0
    for c in range(NCORES):
        total += res.results[c]["acc"].astype(np.float64).sum()
    btot = NCORES * S
    loss = -2.0 * btot - (2.0 / N) * total
    return np.float32(loss)
